# revision 46
# baseline (speedup 1.0000x reference)
"""Depthwise 8192-tap temporal conv (NoRollCaTentLayer) on 8 TRN2 cores, v3.

Per-channel correlation via half-spectrum matmul-FFT (L=8192 = 64-DFT x
twiddle x 128-DFT, keeping k2 in [0,32] of 64 by Hermitian symmetry).

v3 over v2:
 - EI mask (+-1 per 128-ch core slice) applied host-side at gather.
 - F2 stage channel-batched: 4-ch (x) / 2-ch (w) data blocks as one
   [128,128] PE stationary against block-diagonal DFT moving operands
   (12 matmuls/round instead of 32).
 - w-norm path on a [128, CH, 64] layout (full-partition DVE ops)
   instead of [64, CH, 128].
 - I2 matmuls merged across the 4 q-blocks (6 matmuls instead of 12);
   inverse twiddle done in 6 all-q DVE ops instead of 12; k2=32 column
   rides along in a flat [128, 528] pointwise-product layout.
 - Consumer-first emission order (back2, back1, front per iteration),
   prelude loads + F2 constants dispatched ahead of the other constant
   DMAs (split across the sync and Act queues), and the Sqrt activation
   table pre-warmed during idle startup.
"""

import os
import sys

sys.path.insert(0, "/opt/trn_rl_repo")

import numpy as np
import ml_dtypes

import concourse.bacc as bacc
import concourse.mybir as mybir
import concourse.tile as tile
from concourse.bass_utils import run_bass_kernel_spmd

T, C, FW, L = 4096, 1024, 8192, 8192
NUM_INH = 256
EPS = 1e-8
NCORES = 8
CPC = C // NCORES          # 128 channels per core
CH = 16                    # channels per round
R = CPC // CH              # 8 rounds
K2 = 33                    # kept half-spectrum columns (k2 = 0..32)
K2M = 32                   # main columns (k2 < 32)

F32 = mybir.dt.float32
BF16 = mybir.dt.bfloat16
BF = ml_dtypes.bfloat16


def _consts():
    a_ = np.arange(128)
    b32 = np.arange(32)
    b64 = np.arange(64)
    k2_ = np.arange(K2)
    k1_ = np.arange(128)
    u_ = np.arange(128)
    v_ = np.arange(32)

    def pk(mat_re, mat_im):  # [rows, 2, cols] (p-major planes)
        return np.stack([mat_re, mat_im], axis=1)

    th2x = 2 * np.pi * np.outer(b32, k2_) / 64
    f2xs = pk(np.cos(th2x), -np.sin(th2x)).reshape(32, 2 * K2)      # [32, 66]
    th2w = 2 * np.pi * np.outer(b64, k2_) / 64
    f2ws = pk(np.cos(th2w), -np.sin(th2w)).reshape(64, 2 * K2)      # [64, 66]

    # block-diagonal moving operands for the channel-batched F2 stage
    f2xb = np.zeros((128, 4, 2 * K2), np.float32)                   # [(c,m),c',j]
    for c in range(4):
        f2xb[32 * c:32 * c + 32, c] = f2xs
    f2wb = np.zeros((128, 2, 2 * K2), np.float32)
    for c in range(2):
        f2wb[64 * c:64 * c + 64, c] = f2ws
    f2xb = f2xb.astype(BF)
    f2wb = f2wb.astype(BF)

    thtw = 2 * np.pi * np.outer(a_, k2_) / L
    twb0 = np.broadcast_to(np.cos(thtw)[:, None, :, None],
                           (128, 2, K2, 32)).astype(BF)
    twb1 = np.broadcast_to(-np.sin(thtw)[:, None, :, None],
                           (128, 2, K2, 32)).astype(BF)

    th1 = 2 * np.pi * np.outer(a_, k1_) / 128
    # planes: [cos, sin, -sin]
    f1c = np.stack([np.cos(th1), np.sin(th1), -np.sin(th1)],
                   axis=1).astype(BF)                               # [128,3,128]

    thi1 = 2 * np.pi * np.outer(k1_, u_) / 128
    i1cs = np.stack([np.cos(thi1), np.sin(thi1)], axis=1).astype(BF)  # [128,2,128]
    i1cs2 = np.stack([-np.sin(thi1), np.cos(thi1)], axis=1).astype(BF)

    # itwb: partitions (c4, k2<32), free u : e^{+2pi i u k2 / L}
    k2p = np.tile(np.arange(K2M), 4)                                # [128]
    thit = 2 * np.pi * np.outer(k2p, u_) / L
    itwb = np.stack([np.cos(thit), np.sin(thit)], axis=1).astype(BF)  # [128,2,128]
    itwbs = np.stack([np.sin(thit), np.cos(thit)], axis=1).astype(BF)

    # k2=32 inverse movers with the e^{+2pi i u 32/L}/L factor folded in:
    # j1a = cos(2pi(2k1+1)u/256)/L, j1b = -sin(2pi(2k1+1)u/256)/L
    thj = 2 * np.pi * np.outer(2 * k1_ + 1, u_) / 256
    j1a = (np.cos(thj) / L).astype(BF)                              # [128,128]
    j1b = (-np.sin(thj) / L).astype(BF)

    # iw32: partitions c16, free u: e^{+2pi i u 32 / L} / L  (scale folded)
    th32 = 2 * np.pi * u_ * 32 / L
    iw32 = np.stack([np.broadcast_to(np.cos(th32) / L, (16, 128)),
                     np.broadcast_to(np.sin(th32) / L, (16, 128))],
                    axis=1).astype(BF)                              # [16,2,128]

    # I2 block-diagonal stationaries [128=(c4,k2 32), 128=(c4,v 32)]
    wt = np.ones(K2M)
    wt[1:] = 2.0
    cmat = (wt[:, None] * np.cos(2 * np.pi * np.outer(np.arange(K2M), v_) / 64)
            / L)                                                    # [k2, v]
    smat = (wt[:, None] * np.sin(2 * np.pi * np.outer(np.arange(K2M), v_) / 64)
            / L)
    i2c = np.zeros((128, 128), np.float32)
    i2sn = np.zeros((128, 128), np.float32)
    for c in range(4):
        i2c[c * 32:(c + 1) * 32, c * 32:(c + 1) * 32] = cmat
        i2sn[c * 32:(c + 1) * 32, c * 32:(c + 1) * 32] = -smat
    i2c = i2c.astype(BF)
    i2sn = i2sn.astype(BF)
    i2cn = -i2c

    # s32c [16, 4, 128]: for chunk q: S[c16, (c4,v)] = delta_{c,4q+c4} * (-1)^v
    s32c = np.zeros((16, 4, 128), np.float32)
    pmv = ((-1.0) ** v_)
    for q in range(4):
        for c4 in range(4):
            s32c[4 * q + c4, q, c4 * 32:(c4 + 1) * 32] = pmv
    s32c = s32c.astype(BF)

    sel128c = np.zeros((128, 16), np.float32)   # [(c,j8), c'] = delta
    for c in range(16):
        sel128c[8 * c:8 * c + 8, c] = 1.0
    sel128c = sel128c.astype(BF)

    selp = np.zeros((16, 128), np.float32)
    qmask = np.zeros((16, 4), np.float32)
    for j in range(16):
        selp[j, (j % 4) * 32:(j % 4 + 1) * 32] = 1.0
        qmask[j, j // 4] = 1.0
    return {
        "f2xb": f2xb, "f2wb": f2wb, "twb0": twb0, "twb1": twb1, "f1c": f1c,
        "i1cs": i1cs, "i1cs2": i1cs2, "itwb": itwb, "itwbs": itwbs, "iw32": iw32, "j1a": j1a, "j1b": j1b, "i2c": i2c,
        "i2sn": i2sn, "i2cn": i2cn, "s32c": s32c, "selp": selp.astype(BF), "qmask": qmask,
        "sel128c": sel128c,
    }


def _build():
    nc = bacc.Bacc("TRN2", target_bir_lowering=False, debug=False,
                   num_devices=NCORES)
    xp_d = nc.dram_tensor("xprep", [R, 128, 4, 128], BF16, kind="ExternalInput")
    wp_d = nc.dram_tensor("wprep", [R, 128, 8, 128], BF16, kind="ExternalInput")
    wn_d = nc.dram_tensor("wnp", [R, 128, 1024], BF16, kind="ExternalInput")
    f2xb_d = nc.dram_tensor("f2xb", [128, 4, 2 * K2], BF16, kind="ExternalInput")
    f2wb_d = nc.dram_tensor("f2wb", [128, 2, 2 * K2], BF16, kind="ExternalInput")
    twb0_d = nc.dram_tensor("twb0", [128, 2, K2, 32], BF16, kind="ExternalInput")
    twb1_d = nc.dram_tensor("twb1", [128, 2, K2, 32], BF16, kind="ExternalInput")
    f1c_d = nc.dram_tensor("f1c", [128, 3, 128], BF16, kind="ExternalInput")
    i1cs_d = nc.dram_tensor("i1cs", [128, 2, 128], BF16, kind="ExternalInput")
    i1cs2_d = nc.dram_tensor("i1cs2", [128, 2, 128], BF16, kind="ExternalInput")
    itwb_d = nc.dram_tensor("itwb", [128, 2, 128], BF16, kind="ExternalInput")
    itwbs_d = nc.dram_tensor("itwbs", [128, 2, 128], BF16, kind="ExternalInput")
    iw32_d = nc.dram_tensor("iw32", [16, 2, 128], BF16, kind="ExternalInput")
    j1a_d = nc.dram_tensor("j1a", [128, 128], BF16, kind="ExternalInput")
    j1b_d = nc.dram_tensor("j1b", [128, 128], BF16, kind="ExternalInput")
    i2c_d = nc.dram_tensor("i2c", [128, 128], BF16, kind="ExternalInput")
    i2sn_d = nc.dram_tensor("i2sn", [128, 128], BF16, kind="ExternalInput")
    i2cn_d = nc.dram_tensor("i2cn", [128, 128], BF16, kind="ExternalInput")
    s32c_d = nc.dram_tensor("s32c", [16, 4, 128], BF16, kind="ExternalInput")
    selp_d = nc.dram_tensor("selp", [16, 128], BF16, kind="ExternalInput")
    qmask_d = nc.dram_tensor("qmask", [16, 4], F32, kind="ExternalInput")
    sel128c_d = nc.dram_tensor("sel128c", [128, 16], BF16, kind="ExternalInput")
    bcol_d = nc.dram_tensor("bcol", [128, 32], F32, kind="ExternalInput")
    out_d = nc.dram_tensor("outT", [CPC, T], F32, kind="ExternalOutput")

    RELU = mybir.ActivationFunctionType.Relu
    MUL = mybir.AluOpType.mult

    with tile.TileContext(nc) as tc:
        with (
            tc.tile_pool(name="sb", bufs=1) as sb,
            tc.tile_pool(name="ps", bufs=1, space="PSUM") as pp,
        ):
            # F2 constants dispatch before everything (first use, ~6us in)
            f2xb = sb.tile([128, 4, 2 * K2], BF16, tag="c_f2x")
            nc.sync.dma_start(out=f2xb[:], in_=f2xb_d.ap())
            f2wb = sb.tile([128, 2, 2 * K2], BF16, tag="c_f2w")
            nc.sync.dma_start(out=f2wb[:], in_=f2wb_d.ap())
            # ---- prelude: round-0/1 input loads dispatch next ----
            pre = {}
            for _r in (0, 1):
                xs0 = sb.tile([128, 4, 128], BF16, tag="xs", bufs=4)
                nc.sync.dma_start(out=xs0[:], in_=xp_d.ap()[_r])
                ws0 = sb.tile([128, 8, 128], BF16, tag="ws", bufs=4)
                nc.sync.dma_start(out=ws0[:], in_=wp_d.ap()[_r])
                wn0 = sb.tile([128, 1024], BF16, tag="wn", bufs=8)
                nc.sync.dma_start(out=wn0[:], in_=wn_d.ap()[_r])
                pre[_r] = (xs0, ws0, wn0)

            # ---- remaining front-critical constants on sync ----
            twb0 = sb.tile([128, 2, K2, 32], BF16, tag="c_twb0")
            nc.sync.dma_start(out=twb0[:], in_=twb0_d.ap())
            twb1 = sb.tile([128, 2, K2, 32], BF16, tag="c_twb1")
            nc.sync.dma_start(out=twb1[:], in_=twb1_d.ap())
            f1c = sb.tile([128, 3, 128], BF16, tag="c_f1c")
            nc.sync.dma_start(out=f1c[:], in_=f1c_d.ap())
            sel128c = sb.tile([128, 16], BF16, tag="c_selc")
            nc.sync.dma_start(out=sel128c[:], in_=sel128c_d.ap())
            selp = sb.tile([16, 128], BF16, tag="c_selp")
            nc.sync.dma_start(out=selp[:], in_=selp_d.ap())
            qmask = sb.tile([16, 4], F32, tag="c_qmask")
            nc.sync.dma_start(out=qmask[:], in_=qmask_d.ap())
            # later-stage constants via the Act DMA queue (idle at startup)
            i1cs = sb.tile([128, 2, 128], BF16, tag="c_i1cs")
            nc.scalar.dma_start(out=i1cs[:], in_=i1cs_d.ap())
            i1cs2 = sb.tile([128, 2, 128], BF16, tag="c_i1cs2")
            nc.scalar.dma_start(out=i1cs2[:], in_=i1cs2_d.ap())
            itwb = sb.tile([128, 2, 128], BF16, tag="c_itwb")
            nc.scalar.dma_start(out=itwb[:], in_=itwb_d.ap())
            itwbs = sb.tile([128, 2, 128], BF16, tag="c_itwbs")
            nc.scalar.dma_start(out=itwbs[:], in_=itwbs_d.ap())
            j1a = sb.tile([128, 128], BF16, tag="c_j1a")
            nc.scalar.dma_start(out=j1a[:], in_=j1a_d.ap())
            j1b = sb.tile([128, 128], BF16, tag="c_j1b")
            nc.scalar.dma_start(out=j1b[:], in_=j1b_d.ap())
            i2c = sb.tile([128, 128], BF16, tag="c_i2c")
            nc.scalar.dma_start(out=i2c[:], in_=i2c_d.ap())
            i2sn = sb.tile([128, 128], BF16, tag="c_i2sn")
            nc.scalar.dma_start(out=i2sn[:], in_=i2sn_d.ap())
            i2cn = sb.tile([128, 128], BF16, tag="c_i2cn")
            nc.scalar.dma_start(out=i2cn[:], in_=i2cn_d.ap())
            s32c = sb.tile([16, 4, 128], BF16, tag="c_s32c")
            nc.scalar.dma_start(out=s32c[:], in_=s32c_d.ap())
            beis = sb.tile([128, 32], F32, tag="c_beis")
            nc.scalar.dma_start(out=beis[:], in_=bcol_d.ap())

            # warm the Sqrt activation table while Act is otherwise idle
            warm = sb.tile([1, 1], F32, tag="warm")
            nc.gpsimd.memset(warm[:], 1.0)
            nc.scalar.sqrt(warm[:], warm[:])

            def front(r):
                """loads + relu/norm + F2 + Y-copies + twiddle -> state dict"""
                if r in pre:
                    xs, ws, wn = pre[r]
                else:
                    xs = sb.tile([128, 4, 128], BF16, tag="xs", bufs=4)
                    nc.sync.dma_start(out=xs[:], in_=xp_d.ap()[r])
                    ws = sb.tile([128, 8, 128], BF16, tag="ws", bufs=4)
                    nc.sync.dma_start(out=ws[:], in_=wp_d.ap()[r])
                    wn = sb.tile([128, 1024], BF16, tag="wn", bufs=8)
                    nc.sync.dma_start(out=wn[:], in_=wn_d.ap()[r])
                wq = sb.tile([128, 1024], BF16, tag="wq", bufs=2)
                sums = sb.tile([128, 1], F32, tag="sums", bufs=4)
                nc.vector.scalar_tensor_tensor(wq[:], wn[:], 1.0, wn[:],
                                               op0=MUL, op1=MUL,
                                               accum_out=sums[:])
                sumsb = sb.tile([128, 1], BF16, tag="sumsb", bufs=4)
                nc.vector.tensor_copy(sumsb[:], sums[:])
                nrm2 = pp.tile([CH, 1], F32, tag="sm", bufs=1)
                nc.tensor.matmul(nrm2[:], sel128c[:], sumsb[:],
                                 start=True, stop=True)
                rn16 = sb.tile([CH, 1], F32, tag="rn16", bufs=9)
                nc.scalar.sqrt(rn16[:], nrm2[:])
                nc.vector.tensor_scalar_max(rn16[:], rn16[:], EPS)
                nc.vector.reciprocal(rn16[:], rn16[:])
                rnmat = sb.tile([16, 4], BF16, tag="rnmat", bufs=4)
                nc.vector.tensor_mul(rnmat[:], rn16[:].broadcast_to((16, 4)),
                                     qmask[:])
                rnq = sb.tile([128, 4], F32, tag="rnq", bufs=9)
                rnps = pp.tile([128, 4], F32, tag="sm", bufs=1)
                nc.tensor.matmul(rnps[:], selp[:], rnmat[:],
                                 start=True, stop=True)
                nc.scalar.copy(rnq[:], rnps[:])

                yxc = sb.tile([128, 2, K2, 32], BF16, tag="yxc", bufs=3)
                # x path: 4 batched matmuls (4 channels each)
                for g in range(4):
                    yqx = pp.tile([128, 4, 2, K2], F32, tag="yq", bufs=2)
                    nc.tensor.matmul(yqx[:], xs[:, g, :], f2xb[:],
                                     start=True, stop=True)
                    nc.scalar.copy(yxc[:, :, :, 4 * g:4 * g + 4],
                                   yqx[:].transpose((0, 2, 3, 1)))
                # w path: 8 batched matmuls (2 channels each), 3 per bank
                h0 = 0
                for nh in (3, 3, 2):
                    yqw = pp.tile([128, 3, 2, 2, K2], F32, tag="yq", bufs=2)
                    for j in range(nh):
                        nc.tensor.matmul(yqw[:, j], ws[:, h0 + j, :],
                                         f2wb[:], start=True, stop=True)
                    cs = 16 + 2 * h0
                    dst = yxc[:, :, :, cs:cs + 2 * nh].rearrange(
                        "p a k (j c) -> p a k j c", j=nh)
                    nc.scalar.copy(dst, yqw[:, 0:nh].transpose((0, 3, 4, 1, 2)))
                    h0 += nh

                ypc = sb.tile([128, 2, K2, 32], BF16, tag="ypc", bufs=4)
                ta = sb.tile([128, 2, K2, 32], BF16, tag="tt", bufs=4)
                tb = sb.tile([128, 2, K2, 32], BF16, tag="tt", bufs=4)
                nc.vector.tensor_mul(ta[:], yxc[:], twb0[:])
                nc.vector.tensor_mul(tb[:], yxc[:], twb1[:])
                nc.vector.tensor_sub(ypc[:, 0], ta[:, 0], tb[:, 1])
                nc.vector.tensor_add(ypc[:, 1], tb[:, 0], ta[:, 1])
                return {"ypc": ypc, "rnq": rnq, "rn16": rn16}

            def back1(r, st):
                """F1 + psum copies + pointwise -> P tiles"""
                ypc = st["ypc"]
                xw = []
                for path in range(2):
                    cs = slice(16 * path, 16 * path + 16)
                    xmr = pp.tile([128, K2M, 16], F32, tag="xm", bufs=2)
                    nc.tensor.matmul(xmr[:], f1c[:, 0], ypc[:, 0, 0:K2M, cs],
                                     start=True, stop=False)
                    nc.tensor.matmul(xmr[:], f1c[:, 1], ypc[:, 1, 0:K2M, cs],
                                     start=False, stop=True)
                    xmi = pp.tile([128, K2M, 16], F32, tag="xm", bufs=2)
                    nc.tensor.matmul(xmi[:], f1c[:, 0], ypc[:, 1, 0:K2M, cs],
                                     start=True, stop=False)
                    nc.tensor.matmul(xmi[:], f1c[:, 2], ypc[:, 0, 0:K2M, cs],
                                     start=False, stop=True)
                    x32 = pp.tile([128, 2, 16], F32, tag="sm", bufs=1)
                    nc.tensor.matmul(x32[:, 0], f1c[:, 0], ypc[:, 0, K2M, cs],
                                     start=True, stop=False)
                    nc.tensor.matmul(x32[:, 0], f1c[:, 1], ypc[:, 1, K2M, cs],
                                     start=False, stop=True)
                    nc.tensor.matmul(x32[:, 1], f1c[:, 0], ypc[:, 1, K2M, cs],
                                     start=True, stop=False)
                    nc.tensor.matmul(x32[:, 1], f1c[:, 2], ypc[:, 0, K2M, cs],
                                     start=False, stop=True)
                    # flat layout: cols 0:512 = (ch, k2<32), 512:528 = k2=32
                    xsb = sb.tile([128, 2, 528], BF16, tag="xsb", bufs=4)
                    nc.scalar.copy(
                        xsb[:, 0, 0:512].rearrange("p (c k) -> p c k", c=16),
                        xmr[:].transpose((0, 2, 1)))
                    nc.scalar.copy(
                        xsb[:, 1, 0:512].rearrange("p (c k) -> p c k", c=16),
                        xmi[:].transpose((0, 2, 1)))
                    nc.scalar.copy(xsb[:, :, 512:528], x32[:])
                    xw.append(xsb)
                Xs, Ws = xw

                # pointwise product over all 528 cols (k2=32 block rides along)
                Pre = sb.tile([128, 528], BF16, tag="Pre", bufs=3)
                Pim = sb.tile([128, 528], BF16, tag="Pim", bufs=3)
                xm_ = [Xs[:, p] for p in range(2)]
                wm_ = [Ws[:, p] for p in range(2)]
                pp1 = sb.tile([128, 528], BF16, tag="pp", bufs=4)
                pp2 = sb.tile([128, 528], BF16, tag="pp", bufs=4)
                nc.vector.tensor_mul(pp1[:], xm_[0], wm_[0])
                nc.vector.tensor_mul(pp2[:], xm_[1], wm_[1])
                nc.vector.tensor_add(Pre[:], pp1[:], pp2[:])
                pp3 = sb.tile([128, 528], BF16, tag="pp", bufs=4)
                pp4 = sb.tile([128, 528], BF16, tag="pp", bufs=4)
                nc.vector.tensor_mul(pp3[:], xm_[1], wm_[0])
                nc.vector.tensor_mul(pp4[:], xm_[0], wm_[1])
                nc.vector.tensor_sub(Pim[:], pp3[:], pp4[:])
                return {"Pre": Pre, "Pim": Pim}

            def back2(r, mid, st):
                """G/itw + G32 + I2 + activation + store"""
                Pre, Pim = mid["Pre"], mid["Pim"]
                rnq, rn16 = st["rnq"], st["rn16"]
                gts4 = sb.tile([128, 4, 2, 128], BF16, tag="gts", bufs=2)
                for pair in range(2):
                    gt = pp.tile([128, 2, 2, 128], F32, tag="gt", bufs=2)
                    for h in range(2):
                        q = 2 * pair + h
                        nc.tensor.matmul(gt[:, h],
                                         Pre[:, 128 * q:128 * q + 128],
                                         i1cs[:], start=True, stop=False)
                        nc.tensor.matmul(gt[:, h],
                                         Pim[:, 128 * q:128 * q + 128],
                                         i1cs2[:], start=False, stop=True)
                    nc.scalar.copy(gts4[:, 2 * pair:2 * pair + 2], gt[:])

                g32p = pp.tile([16, 128], F32, tag="sm", bufs=1)
                nc.tensor.matmul(g32p[:], Pre[:, 512:528], j1a[:],
                                 start=True, stop=False)
                nc.tensor.matmul(g32p[:], Pim[:, 512:528], j1b[:],
                                 start=False, stop=True)
                gt32 = sb.tile([16, 128], BF16, tag="gt32", bufs=2)
                nc.scalar.copy(gt32[:], g32p[:])


                # inverse twiddle, all 4 q-blocks at once
                iwr = itwb[:, 0].unsqueeze(1).broadcast_to((128, 4, 128))
                iwi = itwb[:, 1].unsqueeze(1).broadcast_to((128, 4, 128))
                gq1 = sb.tile([128, 4, 128], BF16, tag="gq", bufs=6)
                gq2 = sb.tile([128, 4, 128], BF16, tag="gq", bufs=6)
                nc.vector.tensor_mul(gq1[:], gts4[:, :, 0], iwr)
                nc.vector.tensor_mul(gq2[:], gts4[:, :, 1], iwi)
                gq3 = sb.tile([128, 4, 128], BF16, tag="gq", bufs=6)
                gq4 = sb.tile([128, 4, 128], BF16, tag="gq", bufs=6)
                nc.vector.tensor_mul(gq3[:], gts4[:, :, 0], iwi)
                nc.vector.tensor_mul(gq4[:], gts4[:, :, 1], iwr)

                # itw combine folded into I2: i2c@(gq1-gq2) + i2sn@(gq3+gq4)
                ot = pp.tile([128, 4, 128], F32, tag="o", bufs=1)
                nc.tensor.matmul(ot[:], i2c[:], gq1[:],
                                 start=True, stop=False)
                nc.tensor.matmul(ot[:], i2cn[:], gq2[:],
                                 start=False, stop=False)
                nc.tensor.matmul(ot[:], i2sn[:], gq3[:],
                                 start=False, stop=False)
                nc.tensor.matmul(ot[:], i2sn[:], gq4[:],
                                 start=False, stop=False)
                for q in range(4):
                    nc.tensor.matmul(ot[:, q], s32c[:, q], gt32[:],
                                     start=False, stop=True)
                outm = sb.tile([128, 4, 128], F32, tag="outm", bufs=2)
                for q in range(4):
                    j = 4 * r + q
                    nc.scalar.activation(outm[:, q], ot[:, q], RELU,
                                         scale=rnq[:, q:q + 1],
                                         bias=beis[:, j:j + 1])
                nc.sync.dma_start(
                    out=out_d.ap()[CH * r:CH * r + 16].rearrange(
                        "(q c) (v u) -> (c v) q u", q=4, v=32),
                    in_=outm[:])

            # 3-stage software pipeline:
            # F(0) F(1) B1(0) | F(2) B2(0) B1(1) | F(3) B2(1) B1(2) | ...
            sts = {0: front(0), 1: front(1)}
            mids = {0: back1(0, sts[0])}
            for r in range(R):
                back2(r, mids[r], sts[r])
                if r + 1 < R:
                    mids[r + 1] = back1(r + 1, sts[r + 1])
                if r + 2 < R:
                    sts[r + 2] = front(r + 2)

    nc.compile()
    return nc


_CACHE = {}


def _prep(x, w, b):
    """Host-side sharding + layout prep (per core)."""
    consts = _CACHE["consts"]
    in_maps = []
    for i in range(NCORES):
        sl = slice(CPC * i, CPC * (i + 1))
        xs = x[:, sl]                        # [T, CPC]
        wsl = w[:, sl]                       # [FW, CPC]
        # circular arrangement wcr[m]: m<4096: w[m+4095]; m==4096: w[8191];
        # m>4096: w[m-4097]   (slot 4096 unused by conv; holds w[8191] so
        # that sum(wcr^2) == ||w||^2 exactly)
        wcr = np.empty((L, CPC), np.float32)
        wcr[:T] = wsl[T - 1:2 * T - 1]
        wcr[T] = wsl[2 * T - 1]
        wcr[T + 1:] = wsl[0:T - 1]
        # wnp (norm) needs unclamped w; the F2 stationary wants the
        # pos-constrained filter, and max(w,0)/||w|| == max(w/||w||,0)
        wcl = np.maximum(wcr, 0.0)
        # xprep [R, 128=(c4,m32), 4=g, 128=b]: ch = 16r + 4g + c4, t = 128m + b
        xprep = np.ascontiguousarray(
            xs.reshape(32, 128, R, 4, 4).transpose(2, 4, 0, 3, 1)
            .reshape(R, 128, 4, 128).astype(BF))
        # wprep [R, 128=(c2,m64), 8=h, 128=b]: ch = 16r + 2h + c2
        wprep = np.ascontiguousarray(
            wcl.reshape(64, 128, R, 8, 2).transpose(2, 4, 0, 3, 1)
            .reshape(R, 128, 8, 128).astype(BF))
        # wnp [R, 128=(c,j8), 1024]: partition (8c+j) holds chunk j of
        # channel CH*r+c; on-chip square+accum gives per-partition sums
        wnp = np.ascontiguousarray(
            wcr.reshape(8, 1024, R, CH).transpose(2, 3, 0, 1)
            .reshape(R, 128, 1024).astype(BF))
        # bcol [128=(c4,v), 32=j]: bias for channel CH*rr + 4q + c4, j=4rr+q
        bcol = np.zeros((128, 32), np.float32)
        bsl = b[sl]
        for j in range(32):
            rr, q = j // 4, j % 4
            for c4 in range(4):
                ch = CH * rr + 4 * q + c4
                bcol[c4 * 32:(c4 + 1) * 32, j] = bsl[ch]
        m = {"xprep": xprep, "wprep": wprep, "wnp": wnp, "bcol": bcol}
        m.update(consts)
        in_maps.append(m)
    return in_maps


def kernel(x, w, b):
    if "nc" not in _CACHE:
        _CACHE["consts"] = _consts()
        _CACHE["nc"] = _build()
    nc = _CACHE["nc"]

    x = np.ascontiguousarray(np.asarray(x, dtype=np.float32))
    w = np.ascontiguousarray(np.asarray(w, dtype=np.float32))
    b = np.ascontiguousarray(np.asarray(b, dtype=np.float32))
    in_maps = _prep(x, w, b)

    trace = os.environ.get("BASS_CONV_TRACE", "") == "1"
    last_err = None
    for _attempt in range(3):
        try:
            res = run_bass_kernel_spmd(nc, in_maps, core_ids=list(range(NCORES)),
                                       trace=trace)
            break
        except Exception as e:   # transient NRT device errors happen under axon
            last_err = e
    else:
        raise last_err
    if trace and res.exec_time_ns is not None:
        print(f"HW exec time: {res.exec_time_ns} ns")
        kernel.last_exec_ns = res.exec_time_ns
    # EI mask: +1 for excitatory, -1 for the last NUM_INH inhibitory units —
    # constant per 128-channel core slice, applied here at gather.
    ei = np.concatenate([np.ones(C - NUM_INH, np.float32),
                         -np.ones(NUM_INH, np.float32)])
    outs = []
    for i in range(NCORES):
        o = np.asarray(res.results[i]["outT"])          # [CPC, T]
        sgn = ei[CPC * i]                               # whole slice is +-1
        outs.append(np.ascontiguousarray((o * sgn).T))
    return np.concatenate(outs, axis=1)
